# revision 3
# baseline (speedup 1.0000x reference)
"""Causal multi-head self-attention on 8 Trainium2 NeuronCores.

Sharding: core c = (b, g) with b = c // 4 (batch), g = c % 4 (head group).
Each core computes 4 of the 16 heads for one batch element:
  Q/K/V projections for feature rows 256g:256g+256 (Megatron column split),
  causal attention for those heads, and a partial output projection
  against Wo[:, 256g:256g+256] (row split). Host sums the 4 partials per batch.

All operands are pre-transposed on the host so the kernel never transposes:
  xt  = X[b].T          [D, S]   (d on partitions -> matmul contraction dim)
  wqt = Wq[rows].T      [D, 256]
  wkt = Wk[rows].T      [D, 256]
  wvt = Wv[rows].T      [D, 256]
  wot = Wo[:, cols].T   [256, D]

Attention is computed with scores transposed (S^T = K Q^T, kv on partitions)
so the PV matmul needs no transpose, and a ones-row appended to V yields the
softmax denominator inside the same accumulation.
"""

import sys

sys.path.insert(0, "/opt/trn_rl_repo")

import numpy as np

B = 2
S = 2048
D = 1024
H = 16
DH = 64

NCORES = 8
GROUPS = 4            # head groups (cores per batch element)
HPC = H // GROUPS     # heads per core = 4
F = HPC * DH          # feature slice per core = 256

_nc_cache = {}


def _build(s=S):
    import concourse.bass as bass  # noqa: F401
    import concourse.mybir as mybir
    import concourse.tile as tile
    from concourse import bacc

    f32 = mybir.dt.float32
    f32r = mybir.dt.float32r

    P = 128
    SB = 512               # q-block / free-dim block
    NSB = s // SB          # q blocks
    KC = D // P            # 8 contraction chunks over D
    MC = F // P            # 2 feature chunks per core
    NSC = s // P           # s chunks of 128
    ND = D // SB           # 2 output column blocks

    nc = bacc.Bacc("TRN2", debug=False, num_devices=NCORES)
    xt = nc.dram_tensor("xt", [D, s], f32r, kind="ExternalInput").ap()
    wqt = nc.dram_tensor("wqt", [D, F], f32r, kind="ExternalInput").ap()
    wkt = nc.dram_tensor("wkt", [D, F], f32r, kind="ExternalInput").ap()
    wvt = nc.dram_tensor("wvt", [D, F], f32r, kind="ExternalInput").ap()
    wot = nc.dram_tensor("wot", [F, D], f32r, kind="ExternalInput").ap()
    y = nc.dram_tensor("y", [s, D], f32, kind="ExternalOutput").ap()

    with tile.TileContext(nc) as tc:
        with (
            tc.tile_pool(name="w", bufs=1) as wpool,
            tc.tile_pool(name="const", bufs=1) as cpool,
            tc.tile_pool(name="xt", bufs=2) as xpool,
            tc.tile_pool(name="qkv", bufs=1) as qkvpool,
            tc.tile_pool(name="pt", bufs=4) as ptpool,
            tc.tile_pool(name="small", bufs=2) as spool,
            tc.tile_pool(name="yst", bufs=3) as ypool,
            tc.tile_pool(name="ps", bufs=1, space="PSUM") as pspool,
        ):
            # --- weights ---
            wq_s = wpool.tile([P, KC, F], f32r, name="wq_s")
            wk_s = wpool.tile([P, KC, F], f32r, name="wk_s")
            wv_s = wpool.tile([P, KC, F], f32r, name="wv_s")
            wo_s = wpool.tile([P, MC, D], f32r, name="wo_s")
            nc.sync.dma_start(wq_s[:], wqt.rearrange("(o p) f -> p o f", p=P))
            nc.sync.dma_start(wk_s[:], wkt.rearrange("(o p) f -> p o f", p=P))
            nc.sync.dma_start(wv_s[:], wvt.rearrange("(o p) f -> p o f", p=P))
            nc.sync.dma_start(wo_s[:], wot.rearrange("(o p) f -> p o f", p=P))

            # --- causal masks for the 4 diagonal-band kv-chunks of a q-block ---
            # masks[r, j, c] = 1.0 if (128*j + r) <= c else 0.0
            masks = cpool.tile([P, 4, SB], f32, name="masks")
            for j in range(4):
                nc.gpsimd.memset(masks[:, j, :], 1.0)
                nc.gpsimd.affine_select(
                    out=masks[:, j, :],
                    in_=masks[:, j, :],
                    compare_op=mybir.AluOpType.is_ge,
                    fill=0.0,
                    base=-P * j,
                    pattern=[[1, SB]],
                    channel_multiplier=-1,
                )

            # --- persistent activations ---
            qt_t = qkvpool.tile([P, MC, s], f32r, name="qt_t")   # Q^T
            kt_t = qkvpool.tile([P, MC, s], f32r, name="kt_t")   # K^T
            v_t = qkvpool.tile([P, NSC, HPC, DH + 1], f32r, name="v_t")  # V | 1
            ot_t = qkvpool.tile([P, MC, s], f32r, name="ot_t")   # attn out ^T
            ones_sb = cpool.tile([P, NSC * HPC], f32, name="ones_sb")
            nc.gpsimd.memset(ones_sb[:], 1.0)
            nc.vector.tensor_copy(
                out=v_t[:, :, :, DH:DH + 1],
                in_=ones_sb.rearrange("p (a b) -> p a b", b=HPC)[:, :, :, None],
            )

            xt_r = xt.rearrange("(o p) s -> p o s", p=P)

            # --- phase 1: projections (per s-block) ---
            for sb in range(NSB):
                xt_tile = xpool.tile([P, KC, SB], f32r, name="xt_tile")
                nc.sync.dma_start(xt_tile[:], xt_r[:, :, sb * SB:(sb + 1) * SB])
                for w_s, dst in ((wq_s, qt_t), (wk_s, kt_t)):
                    for m in range(MC):
                        pp = pspool.tile([P, SB], f32, name="pp", tag="proj", bufs=2)
                        for k in range(KC):
                            nc.tensor.matmul(
                                pp[:],
                                (w_s[:, k, m * P:(m + 1) * P]),
                                (xt_tile[:, k, :]),
                                start=(k == 0),
                                stop=(k == KC - 1),
                            )
                        nc.vector.tensor_copy(
                            out=dst[:, m, sb * SB:(sb + 1) * SB], in_=pp[:]
                        )
                for sc in range(SB // P):
                    pv = pspool.tile([P, SB], f32, name="pv", tag="proj", bufs=2)
                    for k in range(KC):
                        nc.tensor.matmul(
                            pv[:, :F],
                            (xt_tile[:, k, sc * P:(sc + 1) * P]),
                            (wv_s[:, k, :]),
                            start=(k == 0),
                            stop=(k == KC - 1),
                        )
                    nc.vector.tensor_copy(
                        out=v_t[:, sb * 4 + sc, :, 0:DH],
                        in_=pv[:, :F].rearrange("p (h d) -> p h d", d=DH),
                    )

            # --- phase 2: attention (qb outer so phase 3 can start early) ---
            for qb in range(NSB):
                nkv = 4 * (qb + 1)
                for h in range(HPC):
                    prow = (h % MC) * DH
                    mo = h // MC
                    po_t = pspool.tile([DH + 1, SB], f32, name="po_t", tag="o", bufs=2)
                    pts = []
                    for kv in range(nkv):
                        ps = pspool.tile([P, SB], f32, name="ps", tag="s", bufs=3)
                        nc.tensor.matmul(
                            ps[:],
                            (kt_t[prow:prow + DH, mo, kv * P:(kv + 1) * P]),
                            (qt_t[prow:prow + DH, mo, qb * SB:(qb + 1) * SB]),
                            start=True,
                            stop=True,
                        )
                        pt = ptpool.tile([P, SB], f32r, name="pt", bufs=4)
                        nc.scalar.activation(
                            pt[:],
                            ps[:],
                            mybir.ActivationFunctionType.Exp,
                            scale=float(1.0 / np.sqrt(DH)),
                        )
                        j = kv - 4 * qb
                        if j >= 0:
                            nc.vector.tensor_mul(pt[:], pt[:], masks[:, j, :])
                        pts.append(pt)
                        # software-pipelined PV: lag one chunk behind scores
                        if kv >= 1:
                            nc.tensor.matmul(
                                po_t[:],
                                (v_t[:, kv - 1, h, :]),
                                (pts[kv - 1][:]),
                                start=(kv - 1 == 0),
                                stop=False,
                            )
                    nc.tensor.matmul(
                        po_t[:],
                        (v_t[:, nkv - 1, h, :]),
                        (pts[nkv - 1][:]),
                        start=(nkv - 1 == 0),
                        stop=True,
                    )
                    # normalize: row DH of po_t is the softmax denominator
                    dd = spool.tile([1, SB], f32, name="dd", bufs=2)
                    nc.scalar.copy(dd[:], po_t[DH:DH + 1, :])
                    rr = spool.tile([1, SB], f32, name="rr", bufs=2)
                    nc.vector.reciprocal(rr[:], dd[:])
                    rb = spool.tile([DH, SB], f32, name="rb", bufs=2)
                    nc.gpsimd.partition_broadcast(rb[:], rr[:])
                    nc.vector.tensor_mul(
                        ot_t[prow:prow + DH, mo, qb * SB:(qb + 1) * SB],
                        po_t[0:DH, :],
                        rb[:],
                    )

            # --- phase 3: output projection ---
            for sc in range(NSC):
                for nb in range(ND):
                    py = pspool.tile([P, SB], f32, name="py", tag="proj", bufs=2)
                    for o in range(MC):
                        nc.tensor.matmul(
                            py[:],
                            (ot_t[:, o, sc * P:(sc + 1) * P]),
                            (wo_s[:, o, nb * SB:(nb + 1) * SB]),
                            start=(o == 0),
                            stop=(o == MC - 1),
                        )
                    ys = ypool.tile([P, SB], f32, name="ys", bufs=3)
                    nc.vector.tensor_copy(ys[:], py[:])
                    nc.sync.dma_start(
                        y[sc * P:(sc + 1) * P, nb * SB:(nb + 1) * SB], ys[:]
                    )

    nc.compile()
    return nc


def _get_nc(s=S):
    if s not in _nc_cache:
        _nc_cache[s] = _build(s)
    return _nc_cache[s]


def make_in_maps(in_features, Wq, Wk, Wv, Wo):
    """Shard full inputs into 8 per-core input dicts."""
    x = np.asarray(in_features, dtype=np.float32)
    wq = np.asarray(Wq, dtype=np.float32)
    wk = np.asarray(Wk, dtype=np.float32)
    wv = np.asarray(Wv, dtype=np.float32)
    wo = np.asarray(Wo, dtype=np.float32)

    xts = [np.ascontiguousarray(x[b].T) for b in range(B)]
    in_maps = []
    for c in range(NCORES):
        b, g = divmod(c, GROUPS)
        rows = slice(g * F, (g + 1) * F)
        in_maps.append(
            {
                "xt": xts[b],
                "wqt": np.ascontiguousarray(wq[rows, :].T),
                "wkt": np.ascontiguousarray(wk[rows, :].T),
                "wvt": np.ascontiguousarray(wv[rows, :].T),
                "wot": np.ascontiguousarray(wo[:, rows].T),
            }
        )
    return in_maps


def combine_outputs(results):
    """Sum the 4 partial Y per batch element back into [B, S, D]."""
    out = np.zeros((B, S, D), dtype=np.float32)
    for c in range(NCORES):
        b = c // GROUPS
        out[b] += np.asarray(results[c]["y"])
    return out


def kernel(in_features, Wq, Wk, Wv, Wo):
    from concourse import bass_utils

    nc = _get_nc()
    in_maps = make_in_maps(in_features, Wq, Wk, Wv, Wo)
    res = bass_utils.run_bass_kernel_spmd(nc, in_maps, core_ids=list(range(NCORES)))
    return combine_outputs(res.results)


# revision 9
# speedup vs baseline: 1.1233x; 1.1233x over previous
"""Causal multi-head self-attention on 8 Trainium2 NeuronCores.

Sharding: core c = (b, g) with b = c // 4 (batch), g = c % 4 (head group).
Each core computes 4 of the 16 heads for one batch element:
  Q/K/V projections for feature rows 256g:256g+256 (Megatron column split),
  causal attention for those heads, and a partial output projection
  against Wo[:, 256g:256g+256] (row split). Host sums the 4 partials per batch.

All operands are pre-transposed on the host so the kernel never transposes:
  xt  = X[b].T          [D, S]   (d on partitions -> matmul contraction dim)
  wqt = Wq[rows].T      [D, 256]
  wkt = Wk[rows].T      [D, 256]
  wvt = Wv[rows].T      [D, 256]
  wot = Wo[:, cols].T   [256, D]

Attention is computed with scores transposed (S^T = K Q^T, kv on partitions)
so the PV matmul needs no transpose, and a ones-row appended to V yields the
softmax denominator inside the same accumulation.
"""

import sys

sys.path.insert(0, "/opt/trn_rl_repo")

import numpy as np

B = 2
S = 2048
D = 1024
H = 16
DH = 64

NCORES = 8
GROUPS = 4            # head groups (cores per batch element)
HPC = H // GROUPS     # heads per core = 4
F = HPC * DH          # feature slice per core = 256

_nc_cache = {}


def _build(s=S):
    import concourse.bass as bass  # noqa: F401
    import concourse.mybir as mybir
    import concourse.tile as tile
    from concourse import bacc

    f32 = mybir.dt.float32
    f32r = mybir.dt.float32r
    bf16 = mybir.dt.bfloat16
    dmm = bf16  # matmul operand dtype

    P = 128
    SB = 512               # q-block / free-dim block
    NSB = s // SB          # q blocks
    KC = D // P            # 8 contraction chunks over D
    MC = F // P            # 2 feature chunks per core
    NSC = s // P           # s chunks of 128
    ND = D // SB           # 2 output column blocks

    nc = bacc.Bacc("TRN2", debug=False, num_devices=NCORES)
    xt = nc.dram_tensor("xt", [D, s], dmm, kind="ExternalInput").ap()
    wqt = nc.dram_tensor("wqt", [D, F], dmm, kind="ExternalInput").ap()
    wkt = nc.dram_tensor("wkt", [D, F], dmm, kind="ExternalInput").ap()
    wvt = nc.dram_tensor("wvt", [D, F], dmm, kind="ExternalInput").ap()
    wot = nc.dram_tensor("wot", [F, D], dmm, kind="ExternalInput").ap()
    y = nc.dram_tensor("y", [s, D], f32, kind="ExternalOutput").ap()

    with tile.TileContext(nc) as tc:
        with (
            tc.tile_pool(name="w", bufs=1) as wpool,
            tc.tile_pool(name="const", bufs=1) as cpool,
            tc.tile_pool(name="xt", bufs=2) as xpool,
            tc.tile_pool(name="qkv", bufs=1) as qkvpool,
            tc.tile_pool(name="pt", bufs=4) as ptpool,
            tc.tile_pool(name="small", bufs=2) as spool,
            tc.tile_pool(name="yst", bufs=3) as ypool,
            tc.tile_pool(name="ps", bufs=1, space="PSUM") as pspool,
        ):
            # --- weights ---
            wq_s = wpool.tile([P, KC, F], dmm, name="wq_s")
            wk_s = wpool.tile([P, KC, F], dmm, name="wk_s")
            wv_s = wpool.tile([P, KC, F], dmm, name="wv_s")
            wo_s = wpool.tile([P, MC, D], dmm, name="wo_s")
            nc.sync.dma_start(wq_s[:], wqt.rearrange("(o p) f -> p o f", p=P))
            nc.sync.dma_start(wk_s[:], wkt.rearrange("(o p) f -> p o f", p=P))
            nc.sync.dma_start(wv_s[:], wvt.rearrange("(o p) f -> p o f", p=P))
            nc.sync.dma_start(wo_s[:], wot.rearrange("(o p) f -> p o f", p=P))

            # --- persistent activations ---
            qt_t = qkvpool.tile([P, MC, s], dmm, name="qt_t")   # Q^T
            kt_t = qkvpool.tile([P, MC, s], dmm, name="kt_t")   # K^T
            v_t = qkvpool.tile([P, NSC, HPC, DH + 1], dmm, name="v_t")  # V | 1
            ot_t = qkvpool.tile([P, MC, s], dmm, name="ot_t")   # attn out ^T
            ones_sb = cpool.tile([P, NSC * HPC], f32, name="ones_sb")
            nc.gpsimd.memset(ones_sb[:], 1.0)
            nc.vector.tensor_copy(
                out=v_t[:, :, :, DH:DH + 1],
                in_=ones_sb.rearrange("p (a b) -> p a b", b=HPC)[:, :, :, None],
            )

            xt_r = xt.rearrange("(o p) s -> p o s", p=P)

            # --- phase 1: projections (per s-block) ---
            for sb in range(NSB):
                xt_tile = xpool.tile([P, KC, SB], dmm, name="xt_tile")
                nc.sync.dma_start(xt_tile[:], xt_r[:, :, sb * SB:(sb + 1) * SB])
                for w_s, dst in ((wq_s, qt_t), (wk_s, kt_t)):
                    for m in range(MC):
                        pp = pspool.tile([P, SB], f32, name="pp", tag="proj", bufs=2)
                        for k in range(KC):
                            nc.tensor.matmul(
                                pp[:],
                                (w_s[:, k, m * P:(m + 1) * P]),
                                (xt_tile[:, k, :]),
                                start=(k == 0),
                                stop=(k == KC - 1),
                            )
                        nc.vector.tensor_copy(
                            out=dst[:, m, sb * SB:(sb + 1) * SB], in_=pp[:]
                        )
                for sc in range(SB // P):
                    pv = pspool.tile([P, SB], f32, name="pv", tag="proj", bufs=2)
                    for k in range(KC):
                        nc.tensor.matmul(
                            pv[:, :F],
                            (xt_tile[:, k, sc * P:(sc + 1) * P]),
                            (wv_s[:, k, :]),
                            start=(k == 0),
                            stop=(k == KC - 1),
                        )
                    nc.vector.tensor_copy(
                        out=v_t[:, sb * 4 + sc, :, 0:DH],
                        in_=pv[:, :F].rearrange("p (h d) -> p h d", d=DH),
                    )

            # --- phase 2: attention (qb outer so phase 3 can start early) ---
            for qb in range(NSB):
                nkv = 4 * (qb + 1)
                for h in range(HPC):
                    prow = (h % MC) * DH
                    mo = h // MC
                    po_t = pspool.tile([DH + 1, SB], f32, name="po_t", tag="o", bufs=2)
                    pts = []
                    for kv in range(nkv):
                        ps = pspool.tile([P, SB], f32, name="ps", tag="s", bufs=3)
                        nc.tensor.matmul(
                            ps[:],
                            (kt_t[prow:prow + DH, mo, kv * P:(kv + 1) * P]),
                            (qt_t[prow:prow + DH, mo, qb * SB:(qb + 1) * SB]),
                            start=True,
                            stop=True,
                        )
                        pt = ptpool.tile([P, SB], dmm, name="pt", bufs=4)
                        nc.scalar.activation(
                            pt[:],
                            ps[:],
                            mybir.ActivationFunctionType.Exp,
                            scale=float(1.0 / np.sqrt(DH)),
                        )
                        j = kv - 4 * qb
                        if j >= 0:
                            # causal mask: keep pt[r, c] iff 128*j + r <= c
                            nc.gpsimd.affine_select(
                                out=pt[:],
                                in_=pt[:],
                                compare_op=mybir.AluOpType.is_ge,
                                fill=0.0,
                                base=-P * j,
                                pattern=[[1, SB]],
                                channel_multiplier=-1,
                            )
                        pts.append(pt)
                        # software-pipelined PV: lag one chunk behind scores
                        if kv >= 1:
                            nc.tensor.matmul(
                                po_t[:],
                                (v_t[:, kv - 1, h, :]),
                                (pts[kv - 1][:]),
                                start=(kv - 1 == 0),
                                stop=False,
                            )
                    nc.tensor.matmul(
                        po_t[:],
                        (v_t[:, nkv - 1, h, :]),
                        (pts[nkv - 1][:]),
                        start=(nkv - 1 == 0),
                        stop=True,
                    )
                    # normalize via 1/d = exp(-ln d): ln and exp share one
                    # ACT table set, and this keeps the iterative-divide pipe
                    # (3.3us per 512-elem row) out of the loop entirely.
                    ld = spool.tile([1, SB], f32, name="ld", bufs=2)
                    nc.scalar.activation(
                        ld[:], po_t[DH:DH + 1, :], mybir.ActivationFunctionType.Ln
                    )
                    rr = spool.tile([1, SB], f32, name="rr", bufs=2)
                    nc.scalar.activation(
                        rr[:], ld[:], mybir.ActivationFunctionType.Exp, scale=-1.0
                    )
                    rb = spool.tile([DH, SB], f32, name="rb", bufs=2)
                    nc.gpsimd.partition_broadcast(rb[:], rr[:])
                    nc.vector.tensor_mul(
                        ot_t[prow:prow + DH, mo, qb * SB:(qb + 1) * SB],
                        po_t[0:DH, :],
                        rb[:],
                    )

            # --- phase 3: output projection ---
            for sc in range(NSC):
                for nb in range(ND):
                    py = pspool.tile([P, SB], f32, name="py", tag="proj", bufs=2)
                    for o in range(MC):
                        nc.tensor.matmul(
                            py[:],
                            (ot_t[:, o, sc * P:(sc + 1) * P]),
                            (wo_s[:, o, nb * SB:(nb + 1) * SB]),
                            start=(o == 0),
                            stop=(o == MC - 1),
                        )
                    ys = ypool.tile([P, SB], f32, name="ys", bufs=3)
                    nc.vector.tensor_copy(ys[:], py[:])
                    nc.sync.dma_start(
                        y[sc * P:(sc + 1) * P, nb * SB:(nb + 1) * SB], ys[:]
                    )

    nc.compile()
    return nc


def _get_nc(s=S):
    if s not in _nc_cache:
        _nc_cache[s] = _build(s)
    return _nc_cache[s]


def make_in_maps(in_features, Wq, Wk, Wv, Wo):
    """Shard full inputs into 8 per-core input dicts (bf16 operands)."""
    import ml_dtypes
    bf = ml_dtypes.bfloat16
    x = np.asarray(in_features, dtype=np.float32)
    wq = np.asarray(Wq, dtype=np.float32)
    wk = np.asarray(Wk, dtype=np.float32)
    wv = np.asarray(Wv, dtype=np.float32)
    wo = np.asarray(Wo, dtype=np.float32)

    xts = [np.ascontiguousarray(x[b].T) for b in range(B)]
    in_maps = []
    for c in range(NCORES):
        b, g = divmod(c, GROUPS)
        rows = slice(g * F, (g + 1) * F)
        in_maps.append(
            {
                "xt": xts[b].astype(bf),
                "wqt": np.ascontiguousarray(wq[rows, :].T).astype(bf),
                "wkt": np.ascontiguousarray(wk[rows, :].T).astype(bf),
                "wvt": np.ascontiguousarray(wv[rows, :].T).astype(bf),
                "wot": np.ascontiguousarray(wo[:, rows].T).astype(bf),
            }
        )
    return in_maps


def combine_outputs(results):
    """Sum the 4 partial Y per batch element back into [B, S, D]."""
    out = np.zeros((B, S, D), dtype=np.float32)
    for c in range(NCORES):
        b = c // GROUPS
        out[b] += np.asarray(results[c]["y"])
    return out


def kernel(in_features, Wq, Wk, Wv, Wo):
    from concourse import bass_utils

    nc = _get_nc()
    in_maps = make_in_maps(in_features, Wq, Wk, Wv, Wo)
    res = bass_utils.run_bass_kernel_spmd(nc, in_maps, core_ids=list(range(NCORES)))
    return combine_outputs(res.results)


# revision 11
# speedup vs baseline: 1.3370x; 1.1902x over previous
"""Causal multi-head self-attention on 8 Trainium2 NeuronCores.

Sharding: core c = (b, g) with b = c // 4 (batch), g = c % 4 (head group).
Each core computes 4 of the 16 heads for one batch element:
  Q/K/V projections for feature rows 256g:256g+256 (Megatron column split),
  causal attention for those heads, and a partial output projection
  against Wo[:, 256g:256g+256] (row split). Host sums the 4 partials per batch.

All operands are pre-transposed on the host so the kernel never transposes:
  xt  = X[b].T          [D, S]   (d on partitions -> matmul contraction dim)
  wqt = Wq[rows].T      [D, 256]
  wkt = Wk[rows].T      [D, 256]
  wvt = Wv[rows].T      [D, 256]
  wot = Wo[:, cols].T   [256, D]

Attention is computed with scores transposed (S^T = K Q^T, kv on partitions)
so the PV matmul needs no transpose, and a ones-row appended to V yields the
softmax denominator inside the same accumulation.
"""

import sys

sys.path.insert(0, "/opt/trn_rl_repo")

import numpy as np

B = 2
S = 2048
D = 1024
H = 16
DH = 64

NCORES = 8
GROUPS = 4            # head groups (cores per batch element)
HPC = H // GROUPS     # heads per core = 4
F = HPC * DH          # feature slice per core = 256

_nc_cache = {}


def _build(s=S):
    import concourse.bass as bass  # noqa: F401
    import concourse.mybir as mybir
    import concourse.tile as tile
    from concourse import bacc

    # Make Exp and Ln resolve to the single combined ACT table set so the
    # table-load pass emits one load instead of thrashing between the
    # exp-only and ln-only sets (1.28us per reload).
    import concourse.hw_specs as hw_specs
    if not getattr(bacc, "_act_tables_pinned", False):
        _orig_get_tables = bacc.get_activation_tables

        def _pinned_tables(arch):
            tables = _orig_get_tables(arch)
            exp = mybir.ActivationFunctionType.Exp
            ln = mybir.ActivationFunctionType.Ln
            for name, funcs in tables.items():
                if name != "natural_log_exp_and_others":
                    funcs.discard(exp)
                    funcs.discard(ln)
            return tables

        bacc.get_activation_tables = _pinned_tables
        bacc._act_tables_pinned = True

    f32 = mybir.dt.float32
    f32r = mybir.dt.float32r
    bf16 = mybir.dt.bfloat16
    dmm = bf16  # matmul operand dtype

    P = 128
    SB = 512               # q-block / free-dim block
    NSB = s // SB          # q blocks
    KC = D // P            # 8 contraction chunks over D
    MC = F // P            # 2 feature chunks per core
    NSC = s // P           # s chunks of 128
    ND = D // SB           # 2 output column blocks

    nc = bacc.Bacc("TRN2", debug=False, num_devices=NCORES)
    xt = nc.dram_tensor("xt", [D, s], dmm, kind="ExternalInput").ap()
    wqt = nc.dram_tensor("wqt", [D, F], dmm, kind="ExternalInput").ap()
    wkt = nc.dram_tensor("wkt", [D, F], dmm, kind="ExternalInput").ap()
    wvt = nc.dram_tensor("wvt", [D, F], dmm, kind="ExternalInput").ap()
    wot = nc.dram_tensor("wot", [F, D], dmm, kind="ExternalInput").ap()
    y = nc.dram_tensor("y", [s, D], f32, kind="ExternalOutput").ap()

    with tile.TileContext(nc) as tc:
        with (
            tc.tile_pool(name="w", bufs=1) as wpool,
            tc.tile_pool(name="const", bufs=1) as cpool,
            tc.tile_pool(name="xt", bufs=2) as xpool,
            tc.tile_pool(name="qkv", bufs=1) as qkvpool,
            tc.tile_pool(name="pt", bufs=4) as ptpool,
            tc.tile_pool(name="small", bufs=2) as spool,
            tc.tile_pool(name="yst", bufs=3) as ypool,
            tc.tile_pool(name="ps", bufs=1, space="PSUM") as pspool,
        ):
            # --- weights ---
            wq_s = wpool.tile([P, KC, F], dmm, name="wq_s")
            wk_s = wpool.tile([P, KC, F], dmm, name="wk_s")
            wv_s = wpool.tile([P, KC, F], dmm, name="wv_s")
            wo_s = wpool.tile([P, MC, D], dmm, name="wo_s")
            nc.sync.dma_start(wq_s[:], wqt.rearrange("(o p) f -> p o f", p=P))
            nc.sync.dma_start(wk_s[:], wkt.rearrange("(o p) f -> p o f", p=P))
            nc.sync.dma_start(wv_s[:], wvt.rearrange("(o p) f -> p o f", p=P))
            nc.sync.dma_start(wo_s[:], wot.rearrange("(o p) f -> p o f", p=P))

            # --- causal masks for the 4 diagonal-band kv-chunks ---
            # masks[r, j, c] = 1.0 if (128*j + r) <= c else 0.0
            masks = cpool.tile([P, 4, SB], dmm, name="masks")
            for j in range(4):
                nc.gpsimd.memset(masks[:, j, :], 1.0)
                nc.gpsimd.affine_select(
                    out=masks[:, j, :],
                    in_=masks[:, j, :],
                    compare_op=mybir.AluOpType.is_ge,
                    fill=0.0,
                    base=-P * j,
                    pattern=[[1, SB]],
                    channel_multiplier=-1,
                )

            # --- persistent activations ---
            qt_t = qkvpool.tile([P, MC, s], dmm, name="qt_t")   # Q^T
            kt_t = qkvpool.tile([P, MC, s], dmm, name="kt_t")   # K^T
            v_t = qkvpool.tile([P, NSC, HPC, DH + 1], dmm, name="v_t")  # V | 1
            ot_t = qkvpool.tile([P, MC, s], dmm, name="ot_t")   # attn out ^T
            ones_sb = cpool.tile([P, NSC * HPC], f32, name="ones_sb")
            nc.gpsimd.memset(ones_sb[:], 1.0)
            nc.vector.tensor_copy(
                out=v_t[:, :, :, DH:DH + 1],
                in_=ones_sb.rearrange("p (a b) -> p a b", b=HPC)[:, :, :, None],
            )

            xt_r = xt.rearrange("(o p) s -> p o s", p=P)

            # --- phase 1: projections (per s-block) ---
            for sb in range(NSB):
                xt_tile = xpool.tile([P, KC, SB], dmm, name="xt_tile")
                nc.sync.dma_start(xt_tile[:], xt_r[:, :, sb * SB:(sb + 1) * SB])
                for w_s, dst in ((wq_s, qt_t), (wk_s, kt_t)):
                    for m in range(MC):
                        pp = pspool.tile([P, SB], f32, name="pp", tag="proj", bufs=2)
                        for k in range(KC):
                            nc.tensor.matmul(
                                pp[:],
                                (w_s[:, k, m * P:(m + 1) * P]),
                                (xt_tile[:, k, :]),
                                start=(k == 0),
                                stop=(k == KC - 1),
                            )
                        nc.vector.tensor_copy(
                            out=dst[:, m, sb * SB:(sb + 1) * SB], in_=pp[:]
                        )
                for sc in range(SB // P):
                    pv = pspool.tile([P, SB], f32, name="pv", tag="proj", bufs=2)
                    for k in range(KC):
                        nc.tensor.matmul(
                            pv[:, :F],
                            (xt_tile[:, k, sc * P:(sc + 1) * P]),
                            (wv_s[:, k, :]),
                            start=(k == 0),
                            stop=(k == KC - 1),
                        )
                    nc.vector.tensor_copy(
                        out=v_t[:, sb * 4 + sc, :, 0:DH],
                        in_=pv[:, :F].rearrange("p (h d) -> p h d", d=DH),
                    )

            # --- phase 2: attention (qb outer so phase 3 can start early) ---
            for qb in range(NSB):
                nkv = 4 * (qb + 1)
                for h in range(HPC):
                    prow = (h % MC) * DH
                    mo = h // MC
                    po_t = pspool.tile([DH + 1, SB], f32, name="po_t", tag="o", bufs=2)
                    pts = []
                    for kv in range(nkv):
                        ps = pspool.tile([P, SB], f32, name="ps", tag="s", bufs=4)
                        nc.tensor.matmul(
                            ps[:],
                            (kt_t[prow:prow + DH, mo, kv * P:(kv + 1) * P]),
                            (qt_t[prow:prow + DH, mo, qb * SB:(qb + 1) * SB]),
                            start=True,
                            stop=True,
                        )
                        pt = ptpool.tile([P, SB], dmm, name="pt", bufs=4)
                        nc.scalar.activation(
                            pt[:],
                            ps[:],
                            mybir.ActivationFunctionType.Exp,
                            scale=float(1.0 / np.sqrt(DH)),
                        )
                        j = kv - 4 * qb
                        if j >= 0:
                            nc.vector.tensor_mul(pt[:], pt[:], masks[:, j, :])
                        pts.append(pt)
                        # software-pipelined PV: lag one chunk behind scores
                        if kv >= 1:
                            nc.tensor.matmul(
                                po_t[:],
                                (v_t[:, kv - 1, h, :]),
                                (pts[kv - 1][:]),
                                start=(kv - 1 == 0),
                                stop=False,
                            )
                    nc.tensor.matmul(
                        po_t[:],
                        (v_t[:, nkv - 1, h, :]),
                        (pts[nkv - 1][:]),
                        start=(nkv - 1 == 0),
                        stop=True,
                    )
                    # normalize via 1/d = exp(-ln d): ln and exp share one
                    # ACT table set, and this keeps the iterative-divide pipe
                    # (3.3us per 512-elem row) out of the loop entirely.
                    ld = spool.tile([1, SB], f32, name="ld", bufs=2)
                    nc.scalar.activation(
                        ld[:], po_t[DH:DH + 1, :], mybir.ActivationFunctionType.Ln
                    )
                    rr = spool.tile([1, SB], f32, name="rr", bufs=2)
                    nc.scalar.activation(
                        rr[:], ld[:], mybir.ActivationFunctionType.Exp, scale=-1.0
                    )
                    rb = spool.tile([DH, SB], f32, name="rb", bufs=2)
                    nc.gpsimd.partition_broadcast(rb[:], rr[:])
                    nc.vector.tensor_mul(
                        ot_t[prow:prow + DH, mo, qb * SB:(qb + 1) * SB],
                        po_t[0:DH, :],
                        rb[:],
                    )

            # --- phase 3: output projection ---
            for sc in range(NSC):
                for nb in range(ND):
                    py = pspool.tile([P, SB], f32, name="py", tag="proj", bufs=2)
                    for o in range(MC):
                        nc.tensor.matmul(
                            py[:],
                            (ot_t[:, o, sc * P:(sc + 1) * P]),
                            (wo_s[:, o, nb * SB:(nb + 1) * SB]),
                            start=(o == 0),
                            stop=(o == MC - 1),
                        )
                    ys = ypool.tile([P, SB], f32, name="ys", bufs=3)
                    nc.vector.tensor_copy(ys[:], py[:])
                    nc.sync.dma_start(
                        y[sc * P:(sc + 1) * P, nb * SB:(nb + 1) * SB], ys[:]
                    )

    nc.compile()
    return nc


def _get_nc(s=S):
    if s not in _nc_cache:
        _nc_cache[s] = _build(s)
    return _nc_cache[s]


def make_in_maps(in_features, Wq, Wk, Wv, Wo):
    """Shard full inputs into 8 per-core input dicts (bf16 operands)."""
    import ml_dtypes
    bf = ml_dtypes.bfloat16
    x = np.asarray(in_features, dtype=np.float32)
    wq = np.asarray(Wq, dtype=np.float32)
    wk = np.asarray(Wk, dtype=np.float32)
    wv = np.asarray(Wv, dtype=np.float32)
    wo = np.asarray(Wo, dtype=np.float32)

    xts = [np.ascontiguousarray(x[b].T) for b in range(B)]
    in_maps = []
    for c in range(NCORES):
        b, g = divmod(c, GROUPS)
        rows = slice(g * F, (g + 1) * F)
        in_maps.append(
            {
                "xt": xts[b].astype(bf),
                "wqt": np.ascontiguousarray(wq[rows, :].T).astype(bf),
                "wkt": np.ascontiguousarray(wk[rows, :].T).astype(bf),
                "wvt": np.ascontiguousarray(wv[rows, :].T).astype(bf),
                "wot": np.ascontiguousarray(wo[:, rows].T).astype(bf),
            }
        )
    return in_maps


def combine_outputs(results):
    """Sum the 4 partial Y per batch element back into [B, S, D]."""
    out = np.zeros((B, S, D), dtype=np.float32)
    for c in range(NCORES):
        b = c // GROUPS
        out[b] += np.asarray(results[c]["y"])
    return out


def kernel(in_features, Wq, Wk, Wv, Wo):
    from concourse import bass_utils

    nc = _get_nc()
    in_maps = make_in_maps(in_features, Wq, Wk, Wv, Wo)
    res = bass_utils.run_bass_kernel_spmd(nc, in_maps, core_ids=list(range(NCORES)))
    return combine_outputs(res.results)


# revision 12
# speedup vs baseline: 1.6354x; 1.2232x over previous
"""Causal multi-head self-attention on 8 Trainium2 NeuronCores.

Sharding: core c = (b, g) with b = c // 4 (batch), g = c % 4 (head group).
Each core computes 4 of the 16 heads for one batch element:
  Q/K/V projections for feature rows 256g:256g+256 (Megatron column split),
  causal attention for those heads, and a partial output projection
  against Wo[:, 256g:256g+256] (row split). Host sums the 4 partials per batch.

All operands are pre-transposed on the host so the kernel never transposes:
  xt  = X[b].T          [D, S]   (d on partitions -> matmul contraction dim)
  wqt = Wq[rows].T      [D, 256]
  wkt = Wk[rows].T      [D, 256]
  wvt = Wv[rows].T      [D, 256]
  wot = Wo[:, cols].T   [256, D]

Attention is computed with scores transposed (S^T = K Q^T, kv on partitions)
so the PV matmul needs no transpose, and a ones-row appended to V yields the
softmax denominator inside the same accumulation.
"""

import sys

sys.path.insert(0, "/opt/trn_rl_repo")

import numpy as np

B = 2
S = 2048
D = 1024
H = 16
DH = 64

NCORES = 8
GROUPS = 4            # head groups (cores per batch element)
HPC = H // GROUPS     # heads per core = 4
F = HPC * DH          # feature slice per core = 256

_nc_cache = {}


def _build(s=S):
    import concourse.bass as bass  # noqa: F401
    import concourse.mybir as mybir
    import concourse.tile as tile
    from concourse import bacc

    # Make Exp and Ln resolve to the single combined ACT table set so the
    # table-load pass emits one load instead of thrashing between the
    # exp-only and ln-only sets (1.28us per reload).
    import concourse.hw_specs as hw_specs
    if not getattr(bacc, "_act_tables_pinned", False):
        _orig_get_tables = bacc.get_activation_tables

        def _pinned_tables(arch):
            tables = _orig_get_tables(arch)
            exp = mybir.ActivationFunctionType.Exp
            ln = mybir.ActivationFunctionType.Ln
            for name, funcs in tables.items():
                if name != "natural_log_exp_and_others":
                    funcs.discard(exp)
                    funcs.discard(ln)
            return tables

        bacc.get_activation_tables = _pinned_tables
        bacc._act_tables_pinned = True

    f32 = mybir.dt.float32
    f32r = mybir.dt.float32r
    bf16 = mybir.dt.bfloat16
    dmm = bf16  # matmul operand dtype

    P = 128
    SB = 512               # q-block / free-dim block
    NSB = s // SB          # q blocks
    KC = D // P            # 8 contraction chunks over D
    MC = F // P            # 2 feature chunks per core
    NSC = s // P           # s chunks of 128
    ND = D // SB           # 2 output column blocks

    nc = bacc.Bacc("TRN2", debug=False, num_devices=NCORES)
    xt = nc.dram_tensor("xt", [D, s], dmm, kind="ExternalInput").ap()
    wqt = nc.dram_tensor("wqt", [D, F], dmm, kind="ExternalInput").ap()
    wkt = nc.dram_tensor("wkt", [D, F], dmm, kind="ExternalInput").ap()
    wvt = nc.dram_tensor("wvt", [D, F], dmm, kind="ExternalInput").ap()
    wot = nc.dram_tensor("wot", [F, D], dmm, kind="ExternalInput").ap()
    y = nc.dram_tensor("y", [s, D], f32, kind="ExternalOutput").ap()

    with tile.TileContext(nc) as tc:
        with (
            tc.tile_pool(name="w", bufs=1) as wpool,
            tc.tile_pool(name="const", bufs=1) as cpool,
            tc.tile_pool(name="xt", bufs=2) as xpool,
            tc.tile_pool(name="qkv", bufs=1) as qkvpool,
            tc.tile_pool(name="pt", bufs=4) as ptpool,
            tc.tile_pool(name="small", bufs=2) as spool,
            tc.tile_pool(name="yst", bufs=3) as ypool,
            tc.tile_pool(name="ps", bufs=1, space="PSUM") as pspool,
        ):
            # --- weights ---
            wq_s = wpool.tile([P, KC, F], dmm, name="wq_s")
            wk_s = wpool.tile([P, KC, F], dmm, name="wk_s")
            wv_s = wpool.tile([P, KC, F], dmm, name="wv_s")
            wo_s = wpool.tile([P, MC, D], dmm, name="wo_s")
            nc.sync.dma_start(wq_s[:], wqt.rearrange("(o p) f -> p o f", p=P))
            nc.sync.dma_start(wk_s[:], wkt.rearrange("(o p) f -> p o f", p=P))
            nc.sync.dma_start(wv_s[:], wvt.rearrange("(o p) f -> p o f", p=P))
            nc.sync.dma_start(wo_s[:], wot.rearrange("(o p) f -> p o f", p=P))

            # --- causal masks for the 4 diagonal-band kv-chunks ---
            # masks[r, j, c] = 1.0 if (128*j + r) <= c else 0.0
            masks = cpool.tile([P, 4, SB], dmm, name="masks")
            for j in range(4):
                nc.gpsimd.memset(masks[:, j, :], 1.0)
                nc.gpsimd.affine_select(
                    out=masks[:, j, :],
                    in_=masks[:, j, :],
                    compare_op=mybir.AluOpType.is_ge,
                    fill=0.0,
                    base=-P * j,
                    pattern=[[1, SB]],
                    channel_multiplier=-1,
                )

            # --- persistent activations ---
            qt_t = qkvpool.tile([P, MC, s], dmm, name="qt_t")   # Q^T
            kt_t = qkvpool.tile([P, MC, s], dmm, name="kt_t")   # K^T
            v_t = qkvpool.tile([P, NSC, HPC, DH + 1], dmm, name="v_t")  # V | 1
            ot_t = qkvpool.tile([P, MC, s], dmm, name="ot_t")   # attn out ^T
            ones_sb = cpool.tile([P, NSC * HPC], f32, name="ones_sb")
            nc.gpsimd.memset(ones_sb[:], 1.0)
            nc.vector.tensor_copy(
                out=v_t[:, :, :, DH:DH + 1],
                in_=ones_sb.rearrange("p (a b) -> p a b", b=HPC)[:, :, :, None],
            )

            xt_r = xt.rearrange("(o p) s -> p o s", p=P)

            # --- phase 1: projections (per s-block) ---
            for sb in range(NSB):
                xt_tile = xpool.tile([P, KC, SB], dmm, name="xt_tile")
                nc.sync.dma_start(xt_tile[:], xt_r[:, :, sb * SB:(sb + 1) * SB])
                for w_s, dst in ((wq_s, qt_t), (wk_s, kt_t)):
                    for m in range(MC):
                        pp = pspool.tile([P, SB], f32, name="pp", tag="proj", bufs=2)
                        for k in range(KC):
                            nc.tensor.matmul(
                                pp[:],
                                (w_s[:, k, m * P:(m + 1) * P]),
                                (xt_tile[:, k, :]),
                                start=(k == 0),
                                stop=(k == KC - 1),
                            )
                        nc.vector.tensor_copy(
                            out=dst[:, m, sb * SB:(sb + 1) * SB], in_=pp[:]
                        )
                for sc in range(SB // P):
                    pv = pspool.tile([P, SB], f32, name="pv", tag="proj", bufs=2)
                    for k in range(KC):
                        nc.tensor.matmul(
                            pv[:, :F],
                            (xt_tile[:, k, sc * P:(sc + 1) * P]),
                            (wv_s[:, k, :]),
                            start=(k == 0),
                            stop=(k == KC - 1),
                        )
                    nc.vector.tensor_copy(
                        out=v_t[:, sb * 4 + sc, :, 0:DH],
                        in_=pv[:, :F].rearrange("p (h d) -> p h d", d=DH),
                    )

            # --- phase 2: attention (qb outer so phase 3 can start early) ---
            for qb in range(NSB):
                nkv = 4 * (qb + 1)
                for h in range(HPC):
                    prow = (h % MC) * DH
                    mo = h // MC
                    po_t = pspool.tile([DH + 1, SB], f32, name="po_t", tag="o", bufs=2)
                    # kv-chunks processed in pairs: two scores matmuls fill a
                    # 2-bank psum tile, one exp covers both, PV lags one pair.
                    npair = nkv // 2
                    pts = []
                    for kp in range(npair):
                        kva, kvb = 2 * kp, 2 * kp + 1
                        psS = pspool.tile([P, 2, SB], f32, name="psS", tag="s", bufs=2)
                        for idx, kv in ((0, kva), (1, kvb)):
                            nc.tensor.matmul(
                                psS[:, idx, :],
                                (kt_t[prow:prow + DH, mo, kv * P:(kv + 1) * P]),
                                (qt_t[prow:prow + DH, mo, qb * SB:(qb + 1) * SB]),
                                start=True,
                                stop=True,
                            )
                        pt = ptpool.tile([P, 2, SB], dmm, name="pt", bufs=3)
                        nc.scalar.activation(
                            pt[:],
                            psS[:],
                            mybir.ActivationFunctionType.Exp,
                            scale=float(1.0 / np.sqrt(DH)),
                        )
                        j0 = 2 * kp - 4 * qb
                        if j0 >= 0:
                            nc.vector.tensor_mul(
                                pt[:], pt[:], masks[:, j0:j0 + 2, :]
                            )
                        pts.append(pt)
                        if kp >= 1:
                            for idx, kv in ((0, 2 * kp - 2), (1, 2 * kp - 1)):
                                nc.tensor.matmul(
                                    po_t[:],
                                    (v_t[:, kv, h, :]),
                                    (pts[kp - 1][:, idx, :]),
                                    start=(kv == 0),
                                    stop=False,
                                )
                    for idx, kv in ((0, nkv - 2), (1, nkv - 1)):
                        nc.tensor.matmul(
                            po_t[:],
                            (v_t[:, kv, h, :]),
                            (pts[npair - 1][:, idx, :]),
                            start=(kv == 0),
                            stop=(kv == nkv - 1),
                        )
                    # normalize via 1/d = exp(-ln d): ln and exp share one
                    # ACT table set, and this keeps the iterative-divide pipe
                    # (3.3us per 512-elem row) out of the loop entirely.
                    ld = spool.tile([1, SB], f32, name="ld", bufs=2)
                    nc.scalar.activation(
                        ld[:], po_t[DH:DH + 1, :], mybir.ActivationFunctionType.Ln
                    )
                    rr = spool.tile([1, SB], f32, name="rr", bufs=2)
                    nc.scalar.activation(
                        rr[:], ld[:], mybir.ActivationFunctionType.Exp, scale=-1.0
                    )
                    rb = spool.tile([DH, SB], f32, name="rb", bufs=2)
                    nc.gpsimd.partition_broadcast(rb[:], rr[:])
                    nc.vector.tensor_mul(
                        ot_t[prow:prow + DH, mo, qb * SB:(qb + 1) * SB],
                        po_t[0:DH, :],
                        rb[:],
                    )

            # --- phase 3: output projection ---
            for sc in range(NSC):
                for nb in range(ND):
                    py = pspool.tile([P, SB], f32, name="py", tag="proj", bufs=2)
                    for o in range(MC):
                        nc.tensor.matmul(
                            py[:],
                            (ot_t[:, o, sc * P:(sc + 1) * P]),
                            (wo_s[:, o, nb * SB:(nb + 1) * SB]),
                            start=(o == 0),
                            stop=(o == MC - 1),
                        )
                    ys = ypool.tile([P, SB], f32, name="ys", bufs=3)
                    nc.vector.tensor_copy(ys[:], py[:])
                    nc.sync.dma_start(
                        y[sc * P:(sc + 1) * P, nb * SB:(nb + 1) * SB], ys[:]
                    )

    nc.compile()
    return nc


def _get_nc(s=S):
    if s not in _nc_cache:
        _nc_cache[s] = _build(s)
    return _nc_cache[s]


def make_in_maps(in_features, Wq, Wk, Wv, Wo):
    """Shard full inputs into 8 per-core input dicts (bf16 operands)."""
    import ml_dtypes
    bf = ml_dtypes.bfloat16
    x = np.asarray(in_features, dtype=np.float32)
    wq = np.asarray(Wq, dtype=np.float32)
    wk = np.asarray(Wk, dtype=np.float32)
    wv = np.asarray(Wv, dtype=np.float32)
    wo = np.asarray(Wo, dtype=np.float32)

    xts = [np.ascontiguousarray(x[b].T) for b in range(B)]
    in_maps = []
    for c in range(NCORES):
        b, g = divmod(c, GROUPS)
        rows = slice(g * F, (g + 1) * F)
        in_maps.append(
            {
                "xt": xts[b].astype(bf),
                "wqt": np.ascontiguousarray(wq[rows, :].T).astype(bf),
                "wkt": np.ascontiguousarray(wk[rows, :].T).astype(bf),
                "wvt": np.ascontiguousarray(wv[rows, :].T).astype(bf),
                "wot": np.ascontiguousarray(wo[:, rows].T).astype(bf),
            }
        )
    return in_maps


def combine_outputs(results):
    """Sum the 4 partial Y per batch element back into [B, S, D]."""
    out = np.zeros((B, S, D), dtype=np.float32)
    for c in range(NCORES):
        b = c // GROUPS
        out[b] += np.asarray(results[c]["y"])
    return out


def kernel(in_features, Wq, Wk, Wv, Wo):
    from concourse import bass_utils

    nc = _get_nc()
    in_maps = make_in_maps(in_features, Wq, Wk, Wv, Wo)
    res = bass_utils.run_bass_kernel_spmd(nc, in_maps, core_ids=list(range(NCORES)))
    return combine_outputs(res.results)
